# revision 1
# baseline (speedup 1.0000x reference)
"""Cross-attention Trainium2 kernel (8-core data-parallel over batch).

Per-core computation (one batch element per NeuronCore):
  q = x @ Wq; k = ctx @ Wk; v = ctx @ Wv
  attn = softmax((q k^T) / sqrt(dh)); out = attn @ v; y = out @ Wo + bo

Everything on-chip is kept in "transposed" orientation (feature dim on
partitions, tokens on the free dim) so every matmul streams 512-wide
moving operands:
  xT   [qd, tok]    via PE transposes of natural x tiles (bf16)
  qT   [inner, tok] = Wq_chunk^T @ xT            (bf16 in, fp32 accum)
  sT   [ctx, tok]   = kz_h^T @ qT_pair           (kz_h is the per-head kT
                                                  zero-padded to a full
                                                  128-row stationary; the
                                                  other head's rows are 0 so
                                                  a full-contraction matmul
                                                  yields one head's scores)
  e    [ctx, tok]   = exp(sT / 8)                (ACT; max-subtraction not
                                                  needed: |scores/8| <~ 6)
  r                 = per-head column sums of e, written pre-broadcast across
                      64 partitions by half-ones selector matmuls
  outT [dh, tok]    = v_h^T @ e                  (unnormalized)
  outT_norm         = outT * (1/r)               (DVE, fused into the
                                                  PSUM->SBUF copy)
  y    [tok, qd]    = outT^T @ Wo + bo           (natural orientation, bf16
                                                  store; the caller upcasts)

All SBUF matmul operands are bf16: the PE upconverts to FP22 internally and
accumulates fp32 in PSUM, and bf16 enables fast-weight-load for the
128-column stationaries.

DMA strategy: x rides the SWDGE cast-load path (gpsimd), one 1MB cast-load
per token group — the only steady-state load, it overlaps the HWDGE path.
The weights load fp32 over HWDGE as per-128-row-chunk contiguous
descriptors (a single strided load's 2KB descriptors with 256KB address
jumps thrash DRAM) and are cast to bf16 on DVE, so the PE's first GEMM can
start ~10us in instead of waiting for 5.25MB of weights behind the serial
SWDGE queue. y stores in bf16 on the scalar queue, two half-group stores
per group, interleaved with the bias adds to shorten the tail.
"""

import numpy as np

import concourse.bass as bass
import concourse.tile as tile
from concourse import bacc, mybir
from concourse.bass_utils import run_bass_kernel_spmd
from concourse.masks import make_identity

F32 = mybir.dt.float32
BF16 = mybir.dt.bfloat16

B, N, M = 8, 4096, 77
QD, CD, H, DH = 512, 768, 8, 64
INNER = H * DH  # 512
P = 128
S = 512  # token group size
NQC = QD // P  # 4 qd chunks
NCC = CD // P  # 6 cd chunks
NIC = INNER // P  # 4 inner chunks
NTS = S // P  # 4 token sub-tiles per group
SCALE = DH ** -0.5
MP = 128  # context length padded to full partition width (zeros are inert)


def build_kernel(groups: int = N // S):
    nc = bacc.Bacc(None, target_bir_lowering=False, debug=False)

    x_d = nc.dram_tensor("x", [N, QD], F32, kind="ExternalInput")
    ctx_d = nc.dram_tensor("context", [M, CD], F32, kind="ExternalInput")
    wq_d = nc.dram_tensor("Wq", [QD, INNER], F32, kind="ExternalInput")
    wk_d = nc.dram_tensor("Wk", [CD, INNER], F32, kind="ExternalInput")
    wv_d = nc.dram_tensor("Wv", [CD, INNER], F32, kind="ExternalInput")
    wo_d = nc.dram_tensor("Wo", [INNER, QD], F32, kind="ExternalInput")
    bo_d = nc.dram_tensor("bo", [QD], F32, kind="ExternalInput")
    y_d = nc.dram_tensor("y", [N, QD], BF16, kind="ExternalOutput")

    from contextlib import ExitStack

    with tile.TileContext(nc) as tc, ExitStack() as st:
        consts = st.enter_context(tc.tile_pool(name="consts", bufs=1))
        kvp = st.enter_context(tc.tile_pool(name="kv", bufs=1))
        xin = st.enter_context(tc.tile_pool(name="xin", bufs=3))
        xtp = st.enter_context(tc.tile_pool(name="xt", bufs=2))
        qtp = st.enter_context(tc.tile_pool(name="qt", bufs=2))
        expp = st.enter_context(tc.tile_pool(name="expp", bufs=2))
        rcp = st.enter_context(tc.tile_pool(name="rcp", bufs=2))
        outp = st.enter_context(tc.tile_pool(name="outp", bufs=2))
        yp = st.enter_context(tc.tile_pool(name="yp", bufs=2))

        # PSUM budget: 8 banks total.
        ps_tr = st.enter_context(tc.tile_pool(name="ps_tr", bufs=2, space="PSUM"))
        ps_qf = st.enter_context(tc.tile_pool(name="ps_qf", bufs=2, space="PSUM"))
        ps_s = st.enter_context(tc.tile_pool(name="ps_s", bufs=2, space="PSUM"))
        ps_ro = st.enter_context(tc.tile_pool(name="ps_ro", bufs=2, space="PSUM"))

        # ---- x loads: serial SWDGE cast-load queue, one per group ---------------
        def load_x(g):
            x_g = xin.tile([P, NTS, QD], BF16)
            nc.gpsimd.dma_start(
                out=x_g,
                in_=x_d[g * S : (g + 1) * S, :].rearrange("(p t) q -> p t q", p=P),
            )
            return x_g

        x_pre = [load_x(0), load_x(1)]

        # ---- weights: fp32 HWDGE chunked loads (sync ring) + DVE casts ----------
        def load_w(dram, st_tile, chunks):
            for c in range(chunks):
                nc.sync.dma_start(
                    out=st_tile[:, c, :], in_=dram[c * P : (c + 1) * P, :]
                )

        ctx_st = consts.tile([MP, CD], F32)
        nc.vector.memset(ctx_st, 0.0)
        nc.sync.dma_start(out=ctx_st[:M, :], in_=ctx_d[:, :])
        wq_st = consts.tile([P, NQC, INNER], F32)
        load_w(wq_d, wq_st, NQC)
        wk_st = consts.tile([P, NCC, INNER], F32)
        load_w(wk_d, wk_st, NCC)
        wv_st = consts.tile([P, NCC, INNER], F32)
        load_w(wv_d, wv_st, NCC)
        wo_st = consts.tile([P, NIC, QD], F32)
        load_w(wo_d, wo_st, NIC)
        bo_bc = consts.tile([P, QD], F32)
        bo_ap = bo_d.ap()
        nc.sync.dma_start(
            out=bo_bc, in_=bass.AP(bo_ap.tensor, bo_ap.offset, [[0, P], [1, QD]])
        )

        identity = consts.tile([P, P], BF16)
        make_identity(nc, identity)

        # casts in arrival order: ctx early on DVE, wq on ACT (its queue is
        # idle at startup), wk/wv/wo later on DVE after the first xT copies
        ctx_sb = kvp.tile([MP, CD], BF16)
        nc.vector.tensor_copy(out=ctx_sb, in_=ctx_st)
        wq_sb = consts.tile([P, NQC, INNER], BF16)
        nc.scalar.copy(out=wq_sb, in_=wq_st)
        kz = kvp.tile([P, H, MP], BF16)
        nc.vector.memset(kz, 0.0)

        sel2_stage = consts.tile([M, 2, 2, DH], F32)
        nc.vector.memset(sel2_stage, 0.0)
        nc.vector.memset(sel2_stage[:, 0, 0, :], 1.0)
        nc.vector.memset(sel2_stage[:, 1, 1, :], 1.0)
        sel2 = consts.tile([M, 2, 2, DH], BF16)
        nc.vector.tensor_copy(out=sel2, in_=sel2_stage)

        # ---- x transpose / q projection (emitted separately so the PE can
        # run the transposes while Wq is still loading) ---------------------------
        def emit_trans(g):
            x_g = x_pre[g]
            # transpose x tiles: xT[p, c, t*128+j] = x[4j+t, c*128+p] (the
            # coalesced x load interleaves tokens; every downstream stage is
            # columnwise in tokens and the y-store AP inverts it);
            # 4 PE transposes land in one psum bank, one DVE copy per chunk
            xT = xtp.tile([P, NQC, S], BF16)
            for c in range(NQC):
                pt = ps_tr.tile([P, S], BF16, tag="ps_tr")
                for ts in range(NTS):
                    nc.tensor.transpose(
                        pt[:, ts * P : (ts + 1) * P],
                        x_g[:, ts, c * P : (c + 1) * P],
                        identity,
                    )
                nc.vector.tensor_copy(out=xT[:, c, :], in_=pt)
            return xT

        def emit_qproj(g):
            xT = xT_pre[g]
            qT = qtp.tile([P, NIC, S], BF16)
            for ic in range(NIC):
                pq = ps_qf.tile([P, S], F32, tag="ps_qf")
                for c in range(NQC):
                    nc.tensor.matmul(
                        pq,
                        wq_sb[:, c, ic * P : (ic + 1) * P],
                        xT[:, c, :],
                        start=(c == 0),
                        stop=(c == NQC - 1),
                    )
                nc.scalar.copy(out=qT[:, ic, :], in_=pq)
            return qT

        xT_pre = [emit_trans(0)]

        # ---- context projections (tiny) -----------------------------------------
        ctxT = kvp.tile([P, NCC, MP], BF16)
        for cc in range(NCC):
            pt = ps_tr.tile([P, MP], BF16, tag="ps_tr")
            nc.tensor.transpose(pt, ctx_sb[:, cc * P : (cc + 1) * P], identity)
            nc.vector.tensor_copy(out=ctxT[:, cc, :], in_=pt)

        qT_pre = [emit_qproj(0)]

        wk_sb = consts.tile([P, NCC, INNER], BF16)
        nc.vector.tensor_copy(out=wk_sb, in_=wk_st)

        # k projection: per-head kT zero-padded to full 128-row stationary
        for ic in range(NIC):
            pk = ps_qf.tile([P, S], F32, tag="ps_qf")
            for cc in range(NCC):
                nc.tensor.matmul(
                    pk[:, :MP],
                    wk_sb[:, cc, ic * P : (ic + 1) * P],
                    ctxT[:, cc, :],
                    start=(cc == 0),
                    stop=(cc == NCC - 1),
                )
            nc.scalar.copy(out=kz[:DH, 2 * ic, :], in_=pk[:DH, :MP])
            nc.scalar.copy(out=kz[DH:, 2 * ic + 1, :], in_=pk[DH:P, :MP])

        # ---- scores + exp -------------------------------------------------------
        def emit_front(g):
            qT = qT_pre[g]
            exp_g = expp.tile([MP, H, S], BF16)
            for h in range(H):
                ps_sc = ps_s.tile([MP, S], F32, tag="ps_s")
                nc.tensor.matmul(
                    ps_sc, kz[:, h, :], qT[:, h // 2, :], start=True, stop=True
                )
                nc.scalar.activation(
                    out=exp_g[:, h, :],
                    in_=ps_sc,
                    func=mybir.ActivationFunctionType.Exp,
                    scale=SCALE,
                )
            return exp_g

        exp_pre = [emit_front(0)]

        xT_pre.append(emit_trans(1))
        qT_pre.append(emit_qproj(1))

        wv_sb = consts.tile([P, NCC, INNER], BF16)
        nc.vector.tensor_copy(out=wv_sb, in_=wv_st)
        wo_sb = consts.tile([P, NIC, QD], BF16)
        nc.vector.tensor_copy(out=wo_sb, in_=wo_st)

        # v projection (first consumer is emit_back(0), one iteration away)
        v_sb = kvp.tile([MP, INNER], BF16)
        pv = ps_qf.tile([MP, INNER], F32, tag="ps_qf")
        for cc in range(NCC):
            nc.tensor.matmul(
                pv,
                ctxT[:, cc, :],
                wv_sb[:, cc, :],
                start=(cc == 0),
                stop=(cc == NCC - 1),
            )
        nc.vector.tensor_copy(out=v_sb, in_=pv)

        x_pre.append(load_x(2))

        # ---- rowsums / attention-output / final projection ----------------------
        def emit_back(g):
            exp_g = exp_pre[g]
            # broadcast rowsums + reciprocal per pair
            rec_g = rcp.tile([P, H // 2, S], F32)
            for pp in range(H // 2):
                pr = ps_ro.tile([P, S], F32, tag="ps_ro")
                for side in range(2):
                    nc.tensor.matmul(
                        pr,
                        sel2[:, side],
                        exp_g[:M, 2 * pp + side, :],
                        start=(side == 0),
                        stop=(side == 1),
                    )
                nc.vector.reciprocal_approx_fast(out=rec_g[:, pp, :], in_=pr)

            # outT (unnormalized) * (1/r); pair-packed into one bank
            outT = outp.tile([P, NIC, S], BF16)
            for pp in range(H // 2):
                po = ps_ro.tile([P, S], F32, tag="ps_ro")
                for side in range(2):
                    h = 2 * pp + side
                    nc.tensor.matmul(
                        po[side * DH : (side + 1) * DH, :],
                        v_sb[:, h * DH : (h + 1) * DH],
                        exp_g[:, h, :],
                        start=True,
                        stop=True,
                        tile_position=(0, side * DH),
                    )
                nc.vector.tensor_mul(
                    out=outT[:, pp, :], in0=po, in1=rec_g[:, pp, :]
                )

            # final projection + bias; two half-group bf16 stores; pf
            # partition j holds token 4j+ts, the store AP inverts that
            y_g = yp.tile([P, NTS, QD], BF16)
            y_ap = y_d[g * S : (g + 1) * S, :].rearrange("(p t) q -> p t q", p=P)
            for ts in range(NTS):
                pf = ps_qf.tile([P, QD], F32, tag="ps_qf")
                for ic in range(NIC):
                    nc.tensor.matmul(
                        pf,
                        outT[:, ic, ts * P : (ts + 1) * P],
                        wo_sb[:, ic, :],
                        start=(ic == 0),
                        stop=(ic == NIC - 1),
                    )
                nc.vector.tensor_add(out=y_g[:, ts, :], in0=pf, in1=bo_bc)
                if ts % 2 == 1:
                    nc.scalar.dma_start(
                        out=y_ap[:, ts - 1 : ts + 1, :],
                        in_=y_g[:, ts - 1 : ts + 1, :],
                    )

        # ---- software-pipelined main loop ---------------------------------------
        for g in range(1, groups):
            exp_pre.append(emit_front(g))
            if g + 2 < groups:
                x_pre.append(load_x(g + 2))
            if g + 1 < groups:
                xT_pre.append(emit_trans(g + 1))
            emit_back(g - 1)
            if g + 1 < groups:
                qT_pre.append(emit_qproj(g + 1))
        emit_back(groups - 1)

    nc.compile()
    return nc


_CACHE = {}


def _get_nc():
    if "nc" not in _CACHE:
        _CACHE["nc"] = build_kernel()
    return _CACHE["nc"]


def run(inputs, trace=False, **kw):
    nc = _get_nc()
    in_maps = []
    for i in range(B):
        m = {
            "x": np.asarray(inputs["x"][i], dtype=np.float32),
            "context": np.asarray(inputs["context"][i], dtype=np.float32),
            "Wq": np.asarray(inputs["Wq"], dtype=np.float32),
            "Wk": np.asarray(inputs["Wk"], dtype=np.float32),
            "Wv": np.asarray(inputs["Wv"], dtype=np.float32),
            "Wo": np.asarray(inputs["Wo"], dtype=np.float32),
            "bo": np.asarray(inputs["bo"], dtype=np.float32),
        }
        in_maps.append(m)
    res = run_bass_kernel_spmd(nc, in_maps, list(range(B)), trace=trace, **kw)
    out = np.stack(
        [np.asarray(res.results[i]["y"], dtype=np.float32) for i in range(B)],
        axis=0,
    )
    return out, res


def kernel(**inputs):
    out, _ = run(inputs)
    return out



# revision 3
# speedup vs baseline: 1.0145x; 1.0145x over previous
"""Cross-attention Trainium2 kernel (8-core data-parallel over batch).

Per-core computation (one batch element per NeuronCore):
  q = x @ Wq; k = ctx @ Wk; v = ctx @ Wv
  attn = softmax((q k^T) / sqrt(dh)); out = attn @ v; y = out @ Wo + bo

v2 structure: all layout work (transposes, dtype casts, chunk-major weight
packing) happens on the host, so the device does only matmuls + softmax +
evictions:
  xT   [qd, tok]    loaded directly (host pre-transposed, bf16)
  qT   [inner, tok] = Wq_chunk^T @ xT            (bf16 in, fp32 accum)
  sT   [ctx, tok]   = kz_h^T @ qT_pair           (kz_h per-head kT zero-padded
                                                  to a 128-row stationary)
  e    [ctx, tok]   = exp(sT / 8)                (ACT; |scores/8| small enough
                                                  that max-subtraction is not
                                                  needed)
  r    [pair, tok]  = per-head column sums of e via half-ones selector
                      matmuls, written pre-broadcast across 64 partitions
  outT [dh, tok]    = v_h^T @ e, pair-packed into one PSUM bank via
                      tile_position, then * (1/r) on DVE
  y    [tok, qd]    = outT^T @ Wo + bo           (bf16 store; host upcasts)

DMA: x loads are plain bf16 HWDGE loads (gpsimd queue), weights load in
chunk-major bf16 layout (one contiguous descriptor set each, sync queue),
y stores ride the sync queue after startup.
"""

import numpy as np
import ml_dtypes

import concourse.bass as bass
import concourse.tile as tile
from concourse import bacc, mybir
from concourse.bass_utils import run_bass_kernel_spmd

F32 = mybir.dt.float32
BF16 = mybir.dt.bfloat16
NP_BF16 = ml_dtypes.bfloat16

B, N, M = 8, 4096, 77
QD, CD, H, DH = 512, 768, 8, 64
INNER = H * DH  # 512
P = 128
S = 512  # token group size
NQC = QD // P  # 4 qd chunks
NCC = CD // P  # 6 cd chunks
NIC = INNER // P  # 4 inner chunks
NTS = S // P  # 4 token sub-tiles per group
SCALE = DH ** -0.5
MP = 128  # context length padded to full partition width (zeros are inert)


def build_kernel(groups: int = N // S):
    nc = bacc.Bacc(None, target_bir_lowering=False, debug=False)

    xt_d = nc.dram_tensor("xT", [QD, N], BF16, kind="ExternalInput")
    ctxt_d = nc.dram_tensor("ctxT", [CD, MP], BF16, kind="ExternalInput")
    wq_d = nc.dram_tensor("Wq", [P, NQC * INNER], BF16, kind="ExternalInput")
    wk_d = nc.dram_tensor("Wk", [P, NCC * INNER], BF16, kind="ExternalInput")
    wv_d = nc.dram_tensor("Wv", [P, NCC * INNER], BF16, kind="ExternalInput")
    wo_d = nc.dram_tensor("Wo", [P, NIC * QD], BF16, kind="ExternalInput")
    bo_d = nc.dram_tensor("bo", [QD], F32, kind="ExternalInput")
    y_d = nc.dram_tensor("y", [N, QD], BF16, kind="ExternalOutput")

    from contextlib import ExitStack

    with tile.TileContext(nc) as tc, ExitStack() as st:
        consts = st.enter_context(tc.tile_pool(name="consts", bufs=1))
        xin = st.enter_context(tc.tile_pool(name="xin", bufs=3))
        qtp = st.enter_context(tc.tile_pool(name="qt", bufs=2))
        expp = st.enter_context(tc.tile_pool(name="expp", bufs=2))
        rcp = st.enter_context(tc.tile_pool(name="rcp", bufs=2))
        outp = st.enter_context(tc.tile_pool(name="outp", bufs=2))
        yp = st.enter_context(tc.tile_pool(name="yp", bufs=2))

        # PSUM budget: 8 banks total.
        ps_q = st.enter_context(tc.tile_pool(name="ps_q", bufs=2, space="PSUM"))
        ps_s = st.enter_context(tc.tile_pool(name="ps_s", bufs=2, space="PSUM"))
        ps_rs = st.enter_context(tc.tile_pool(name="ps_rs", bufs=2, space="PSUM"))
        ps_av = st.enter_context(tc.tile_pool(name="ps_av", bufs=2, space="PSUM"))

        # ---- weight / context loads (sync queue, needed-first order) ------------
        wq_sb = consts.tile([P, NQC, INNER], BF16)
        nc.sync.dma_start(out=wq_sb, in_=wq_d.rearrange("p (c i) -> p c i", c=NQC))
        wk_sb = consts.tile([P, NCC, INNER], BF16)
        nc.sync.dma_start(out=wk_sb, in_=wk_d.rearrange("p (c i) -> p c i", c=NCC))
        ctxT_sb = consts.tile([P, NCC, MP], BF16)
        nc.sync.dma_start(
            out=ctxT_sb, in_=ctxt_d.rearrange("(c p) m -> p c m", p=P)
        )
        wv_sb = consts.tile([P, NCC, INNER], BF16)
        nc.sync.dma_start(out=wv_sb, in_=wv_d.rearrange("p (c i) -> p c i", c=NCC))
        wo_sb = consts.tile([P, NIC, QD], BF16)
        nc.sync.dma_start(out=wo_sb, in_=wo_d.rearrange("p (c i) -> p c i", c=NIC))
        bo_bc = consts.tile([P, QD], F32)
        bo_ap = bo_d.ap()
        nc.sync.dma_start(
            out=bo_bc, in_=bass.AP(bo_ap.tensor, bo_ap.offset, [[0, P], [1, QD]])
        )

        # ---- x loads: plain bf16 loads, gpsimd queue, one per group -------------
        def load_x(g):
            x_g = xin.tile([P, NQC, S], BF16)
            nc.gpsimd.dma_start(
                out=x_g,
                in_=xt_d.rearrange("(c p) n -> p c n", p=P)[
                    :, :, g * S : (g + 1) * S
                ],
            )
            return x_g

        x_pre = [load_x(0), load_x(1)]

        # rowsum selector stationaries: sel2[:, side] is [M, 128] with ones in
        # columns side*64..(side+1)*64
        sel2_stage = consts.tile([M, 2, 2, DH], F32)
        nc.vector.memset(sel2_stage, 0.0)
        nc.vector.memset(sel2_stage[:, 0, 0, :], 1.0)
        nc.vector.memset(sel2_stage[:, 1, 1, :], 1.0)
        sel2 = consts.tile([M, 2, 2, DH], BF16)
        nc.vector.tensor_copy(out=sel2, in_=sel2_stage)

        kz = consts.tile([P, H, MP], BF16)
        nc.vector.memset(kz, 0.0)

        # ---- q projection -------------------------------------------------------
        def emit_qproj(g):
            xT = x_pre[g]
            qT = qtp.tile([P, NIC, S], BF16)
            for ic in range(NIC):
                pq = ps_q.tile([P, S], F32, tag="ps_q")
                for c in range(NQC):
                    nc.tensor.matmul(
                        pq,
                        wq_sb[:, c, ic * P : (ic + 1) * P],
                        xT[:, c, :],
                        start=(c == 0),
                        stop=(c == NQC - 1),
                    )
                nc.scalar.copy(out=qT[:, ic, :], in_=pq)
            return qT

        qT_pre = [emit_qproj(0)]

        # ---- k projection: per-head kT zero-padded to full 128-row stationary ---
        for ic in range(NIC):
            pk = ps_s.tile([P, MP], F32, tag="ps_s")
            for cc in range(NCC):
                nc.tensor.matmul(
                    pk,
                    wk_sb[:, cc, ic * P : (ic + 1) * P],
                    ctxT_sb[:, cc, :],
                    start=(cc == 0),
                    stop=(cc == NCC - 1),
                )
            nc.scalar.copy(out=kz[:DH, 2 * ic, :], in_=pk[:DH, :])
            nc.scalar.copy(out=kz[DH:, 2 * ic + 1, :], in_=pk[DH:, :])

        # ---- v projection: v_sb [ctx, inner] ------------------------------------
        v_sb = consts.tile([MP, INNER], BF16)
        pv = ps_q.tile([MP, INNER], F32, tag="ps_q")
        for cc in range(NCC):
            nc.tensor.matmul(
                pv,
                ctxT_sb[:, cc, :],
                wv_sb[:, cc, :],
                start=(cc == 0),
                stop=(cc == NCC - 1),
            )
        nc.vector.tensor_copy(out=v_sb, in_=pv)

        x_pre.append(load_x(2))

        # ---- scores + exp -------------------------------------------------------
        def emit_front(g):
            qT = qT_pre[g]
            exp_g = expp.tile([MP, H, S], BF16)
            for h in range(H):
                ps_sc = ps_s.tile([MP, S], F32, tag="ps_s")
                nc.tensor.matmul(
                    ps_sc, kz[:, h, :], qT[:, h // 2, :], start=True, stop=True
                )
                nc.scalar.activation(
                    out=exp_g[:, h, :],
                    in_=ps_sc,
                    func=mybir.ActivationFunctionType.Exp,
                    scale=SCALE,
                )
            return exp_g

        exp_pre = [emit_front(0)]

        # ---- rowsums / attention-output / final projection ----------------------
        def emit_back(g):
            exp_g = exp_pre[g]
            # broadcast rowsums + reciprocal per pair
            rec_g = rcp.tile([P, H // 2, S], F32)
            for pp in range(H // 2):
                pr = ps_rs.tile([P, S], F32, tag="ps_rs")
                for side in range(2):
                    nc.tensor.matmul(
                        pr,
                        sel2[:, side],
                        exp_g[:M, 2 * pp + side, :],
                        start=(side == 0),
                        stop=(side == 1),
                    )
                nc.vector.reciprocal_approx_fast(out=rec_g[:, pp, :], in_=pr)

            # outT (unnormalized) * (1/r); pair-packed into one bank
            outT = outp.tile([P, NIC, S], BF16)
            for pp in range(H // 2):
                po = ps_av.tile([P, S], F32, tag="ps_av")
                for side in range(2):
                    h = 2 * pp + side
                    nc.tensor.matmul(
                        po[side * DH : (side + 1) * DH, :],
                        v_sb[:, h * DH : (h + 1) * DH],
                        exp_g[:, h, :],
                        start=True,
                        stop=True,
                        tile_position=(0, side * DH),
                    )
                nc.vector.tensor_mul(
                    out=outT[:, pp, :], in0=po, in1=rec_g[:, pp, :]
                )

            # final projection + bias; two half-group bf16 stores; pf
            # partition j holds token ts*128+j
            y_g = yp.tile([P, NTS, QD], BF16)
            y_ap = y_d[g * S : (g + 1) * S, :].rearrange("(t p) q -> p t q", p=P)
            for ts in range(NTS):
                pf = ps_q.tile([P, QD], F32, tag="ps_q")
                for ic in range(NIC):
                    nc.tensor.matmul(
                        pf,
                        outT[:, ic, ts * P : (ts + 1) * P],
                        wo_sb[:, ic, :],
                        start=(ic == 0),
                        stop=(ic == NIC - 1),
                    )
                nc.vector.tensor_add(out=y_g[:, ts, :], in0=pf, in1=bo_bc)
                if ts % 2 == 1:
                    nc.sync.dma_start(
                        out=y_ap[:, ts - 1 : ts + 1, :],
                        in_=y_g[:, ts - 1 : ts + 1, :],
                    )

        # ---- software-pipelined main loop ---------------------------------------
        for g in range(1, groups):
            if g + 2 < groups:
                x_pre.append(load_x(g + 2))
            qT_pre.append(emit_qproj(g))
            exp_pre.append(emit_front(g))
            emit_back(g - 1)
        emit_back(groups - 1)

    nc.compile()
    return nc


_CACHE = {}


def _get_nc():
    if "nc" not in _CACHE:
        _CACHE["nc"] = build_kernel()
    return _CACHE["nc"]


def _chunk_major(w, nchunks):
    """[nchunks*128, F] f32 -> [128, nchunks*F] bf16, chunk-major per partition."""
    f = w.shape[1]
    return np.ascontiguousarray(
        w.reshape(nchunks, P, f).transpose(1, 0, 2).reshape(P, nchunks * f)
    ).astype(NP_BF16)


def run(inputs, trace=False, **kw):
    nc = _get_nc()
    wq_h = _chunk_major(np.asarray(inputs["Wq"], np.float32), NQC)
    wk_h = _chunk_major(np.asarray(inputs["Wk"], np.float32), NCC)
    wv_h = _chunk_major(np.asarray(inputs["Wv"], np.float32), NCC)
    wo_h = _chunk_major(np.asarray(inputs["Wo"], np.float32), NIC)
    bo_h = np.asarray(inputs["bo"], np.float32)
    in_maps = []
    for i in range(B):
        xt = np.ascontiguousarray(np.asarray(inputs["x"][i], np.float32).T).astype(
            NP_BF16
        )
        ctx = np.zeros((CD, MP), np.float32)
        ctx[:, :M] = np.asarray(inputs["context"][i], np.float32).T
        in_maps.append(
            {
                "xT": xt,
                "ctxT": ctx.astype(NP_BF16),
                "Wq": wq_h,
                "Wk": wk_h,
                "Wv": wv_h,
                "Wo": wo_h,
                "bo": bo_h,
            }
        )
    res = run_bass_kernel_spmd(nc, in_maps, list(range(B)), trace=trace, **kw)
    out = np.stack(
        [np.asarray(res.results[i]["y"], dtype=np.float32) for i in range(B)],
        axis=0,
    )
    return out, res


def kernel(**inputs):
    out, _ = run(inputs)
    return out
